# revision 3
# baseline (speedup 1.0000x reference)
"""N-pairs custom loss on 8 Trainium2 NeuronCores.

Math
----
reference computes, with a' = anchor + 1e-6:
    sq[i,j] = ||a'_i||^2 + ||p_j||^2 - 2 a'_i . p_j
    dist    = sqrt(max(sq, 1e-12))
    hinge   = relu(diag(dist)[i] + 1 - dist[i,j])
    loss    = sum over {i : label_i == 1, j != i} hinge / count

Device-side restructuring:
  * Only rows with label==1 contribute -> host compacts those rows
    (K ~ N/2), sharded 512 rows/core across 8 cores; a small remainder
    beyond full launches is summed on the host.  512 rows/core keeps the
    PE weight reloads at 4/core (one per row tile) -- finer row sharding
    stalls the PE ~500ns per extra weight swap.
  * The PE emits y = (c_i^2 - sq_ij) / (2 c_i) directly (c_i =
    pos_dist_i + MARGIN): the augmented anchor rows are affinely
    transformed on the host against phat_j = [sqrt(2) p_j, 1, ||p_j||^2]
    (66 contraction dims).  Then u = dist/c = sqrt(1 - 2y/c), so ACT
    computes u = sqrt(y * (-2/c_i) + 1) in one PSUM->SBUF bf16 pass with
    per-partition scale.
  * The DVE reduces sum_j min(u, 1) per row with one tensor_scalar
    (op0=min 1.0 computes the elements, op1=add is the accumulator's
    reduce op).  One tile's reduce runs on ACT instead
    (min(u,1) = u - relu(u-1), two accumulating ACT passes): the DVE
    reduce cadence (~2330ns/tile) is slightly above ACT's (~2060ns), so
    one ACT-offloaded tile levels the two queues.  The diagonal j==i
    contributes exactly MARGIN per row; the host subtracts it.
  * Per-core partials land in a small [128, 20] accumulator; its first
    half is DMA'd out mid-kernel so the final transfer is tiny.
    Host: total = sum_i c_i*(N - msum_i) - K; loss = total/count.

This walrus build accepts only ONE sync wait per instruction; a
post-serialization pass splits excess waits into EventSemaphore
instructions and fuses Ldweights into self-loading matmuls so walrus's
LDW optimization can pipeline weight loads (see _legalize_bir).
"""

import numpy as np

import concourse.bass as bass
import concourse.mybir as mybir
from concourse import tile
from concourse.bass_utils import run_bass_kernel_spmd

N_CORES = 8
NCOLS = 8192              # number of positive embeddings (full N)
D = 64
KAUG = D + 2              # augmented contraction dim
ROW_TILE = 128
N_ROW_TILES = 4
R_PER_CORE = ROW_TILE * N_ROW_TILES      # 512
ROW_CAP = N_CORES * R_PER_CORE           # 4096 label-1 rows per launch
HOST_TAIL_MAX = 256   # rows beyond full launches handled on host (numpy)
COL_CHUNK = 2048                         # 4 PSUM banks
N_COL_CHUNKS = NCOLS // COL_CHUNK        # 4
MM_FREE = 512                            # moving free dim per matmul (PSUM bank)
MARGIN = 1.0
EPS = 1e-6
ACT_TILE = (3, 2)    # the (row-tile, chunk) whose reduce runs on ACT
ACT_SLOT = ACT_TILE[0] * N_COL_CHUNKS + ACT_TILE[1]
NSLOTS = N_ROW_TILES * N_COL_CHUNKS      # 16
N_ACC = NSLOTS + 3   # +2 split-tile halves, +1 ACT tile's relu term
ACC_EARLY = 8        # acc columns DMA'd out mid-kernel

_CACHED_NC = None
last_results = None       # BassKernelResults of the most recent launch
TRACE = False             # set True (e.g. from test.py) to capture a profile
TRACE_CORES = None        # e.g. list(range(8)) to profile all cores


def _build_nc():
    nc = bass.Bass()
    ahat = nc.dram_tensor("ahat", [KAUG, R_PER_CORE], mybir.dt.float16,
                          kind="ExternalInput")
    phat = nc.dram_tensor("phat", [KAUG, NCOLS], mybir.dt.float16,
                          kind="ExternalInput")
    svec = nc.dram_tensor("svec", [ROW_TILE, N_ROW_TILES], mybir.dt.float32,
                          kind="ExternalInput")
    acc_out = nc.dram_tensor("acc", [ROW_TILE, N_ACC],
                             mybir.dt.float32, kind="ExternalOutput")

    with tile.TileContext(nc) as tc:
        with (
            tc.tile_pool(name="const", bufs=1) as const_pool,
            # one SBUF slot per tile's u: slot reuse would force extra
            # ACT/DVE WAR semaphore waits (each costing a split EVSEM)
            tc.tile_pool(name="upool", bufs=NSLOTS + 2) as u_pool,
            tc.tile_pool(name="psum", bufs=2, space="PSUM") as psum_pool,
        ):
            ahat_sb = const_pool.tile([KAUG, R_PER_CORE], mybir.dt.float16)
            phat_sb = const_pool.tile([KAUG, NCOLS], mybir.dt.float16)
            s_sb = const_pool.tile([ROW_TILE, N_ROW_TILES], mybir.dt.float32)
            negone_sb = const_pool.tile([ROW_TILE, 1], mybir.dt.float32)
            junk_sb = const_pool.tile([ROW_TILE, COL_CHUNK], mybir.dt.bfloat16)
            acc_sb = const_pool.tile([ROW_TILE, N_ACC], mybir.dt.float32)
            nc.vector.memset(negone_sb[:], -1.0)

            # preload the sqrt activation-table set (~2.7us) as early as
            # possible on the ACT engine: scale=0.0 means the input value
            # is never consumed, so the only dependency is the tiny negone
            # memset and the table load overlaps the NEFF preamble / DMA
            # window instead of gating the first real sqrt
            warm_out = const_pool.tile([1, 1], mybir.dt.float16)
            nc.scalar.activation(warm_out[:], negone_sb[:1, :1],
                                 mybir.ActivationFunctionType.Sqrt,
                                 scale=0.0)

            # Three DMA queues (sync + scalar are both HWDGE rings, gpsimd
            # is SWDGE).  The first compute tile needs only ahat + the
            # first 512 phat columns: put those two small pieces first on
            # the two fast HWDGE queues, then round-robin the phat bulk.
            nc.sync.dma_start(ahat_sb[:], ahat[:])
            nc.scalar.dma_start(phat_sb[:, 0:512], phat[:, 0:512])
            nc.gpsimd.dma_start(s_sb[:], svec[:])
            nc.gpsimd.dma_start(phat_sb[:, 512:1024], phat[:, 512:1024])
            engines = [nc.sync, nc.scalar, nc.gpsimd]
            piece = 1024
            q = 0
            c0 = 1024
            while c0 < NCOLS:
                c1 = min(NCOLS, c0 + piece)
                engines[q % 3].dma_start(phat_sb[:, c0:c1], phat[:, c0:c1])
                q += 1
                c0 = c1

            def emit_tile(r, col0, clen, slot, on_act=False):
                ps = psum_pool.tile([ROW_TILE, COL_CHUNK], mybir.dt.float32,
                                    tag="ps")
                for k in range(clen // MM_FREE):
                    nc.tensor.matmul(
                        ps[:, k * MM_FREE:(k + 1) * MM_FREE],
                        ahat_sb[:, r * ROW_TILE:(r + 1) * ROW_TILE],
                        phat_sb[:, col0 + k * MM_FREE:col0 + (k + 1) * MM_FREE],
                        start=True, stop=True,
                    )
                u_t = u_pool.tile([ROW_TILE, COL_CHUNK],
                                  mybir.dt.bfloat16, tag="u")
                # u = sqrt(y * (-2/c_i) + 1) = dist/c; for the ACT-offloaded
                # tile also accumulate sum(u) in the same pass
                nc.scalar.activation(u_t[:, :clen], ps[:, :clen],
                                     mybir.ActivationFunctionType.Sqrt,
                                     scale=s_sb[:, r:r + 1], bias=1.0,
                                     accum_out=(acc_sb[:, slot:slot + 1]
                                                if on_act else None))
                if on_act:
                    # min(u,1) = u - relu(u-1) exactly, so this tile's sum
                    # needs only a second ACT pass (relu with bias -1 and
                    # accumulate); the DVE-side reduce is the kernel's
                    # bottleneck so one tile moves engines to balance.
                    nc.scalar.activation(junk_sb[:, :clen], u_t[:, :clen],
                                         mybir.ActivationFunctionType.Relu,
                                         bias=negone_sb[:, 0:1],
                                         accum_out=acc_sb[:, N_ACC - 1:N_ACC])
                else:
                    # accum_out[p] = sum_j min(u, 1): op0 computes the out
                    # elements, op1 is the accumulator's reduce op
                    nc.vector.tensor_scalar(
                        out=junk_sb[:, :clen],
                        in0=u_t[:, :clen],
                        scalar1=1.0, scalar2=0.0,
                        op0=mybir.AluOpType.min,
                        op1=mybir.AluOpType.add,
                        accum_out=acc_sb[:, slot:slot + 1],
                    )

            nc.vector.memset(acc_sb[:], 0.0)
            for r in range(N_ROW_TILES):
                for t in range(N_COL_CHUNKS):
                    slot = r * N_COL_CHUNKS + t
                    first = (r == 0 and t == 0)
                    last = (r == N_ROW_TILES - 1 and t == N_COL_CHUNKS - 1)
                    if first:
                        # 512-wide head tile: starts after one DMA piece
                        emit_tile(r, 0, 512, slot)
                        emit_tile(r, 512, COL_CHUNK - 512, NSLOTS)
                    elif last:
                        # 512-wide final tile: shorter end-of-pipeline drain
                        h = COL_CHUNK - 512
                        emit_tile(r, t * COL_CHUNK, h, slot)
                        emit_tile(r, t * COL_CHUNK + h, 512, NSLOTS + 1)
                    else:
                        emit_tile(r, t * COL_CHUNK, COL_CHUNK, slot,
                                  on_act=((r, t) == ACT_TILE))
                    if slot == 11 and t == 3:
                        # first 8 acc columns (row tiles 0-1) are final by
                        # now: ship them early so the tail transfer is tiny
                        nc.scalar.dma_start(acc_out[:, 0:ACC_EARLY],
                                            acc_sb[:, 0:ACC_EARLY])
            nc.sync.dma_start(acc_out[:, ACC_EARLY:N_ACC],
                              acc_sb[:, ACC_EARLY:N_ACC])
    return nc


def _legalize_bir(bir_bytes):
    """Two fixups on the serialized BIR before walrus:

    1. Fuse each standalone Ldweights into its paired (self-loading)
       Matmult: walrus's LDW optimization (background weight buffer ->
       weight loads overlap in-flight matmuls) rejects standalone
       InstLdweights, and without it every LDW/MM pair serializes at the
       full matmul drain latency (~630ns instead of ~430ns per matmul).

    2. This walrus build accepts only ONE sync wait per instruction (two
       on EventSemaphore); Tile emits more on some (epilogue drain, ...).
       Split excess waits into standalone EventSemaphore wait instructions
       on the same engine, inserted immediately before (semantically
       identical: the engine blocks on the same condition set, in order).
    """
    import json as _json
    m = _json.loads(bir_bytes)
    for fn in m["functions"]:
        for blk in fn["blocks"]:
            out = []
            pending_ld = None
            for ins in blk["instructions"]:
                op = ins.get("opcode")
                if op == "Ldweights":
                    if pending_ld is not None:
                        out.append(pending_ld)
                    pending_ld = ins
                    continue
                if op == "Matmult" and pending_ld is not None:
                    if pending_ld["ins"][0] == ins["ins"][1]:
                        ins["ldweights"] = True
                        lsi = pending_ld.get("sync_info") or {}
                        msi = ins.setdefault("sync_info", {})
                        msi["on_wait"] = list(lsi.get("on_wait") or []) + \
                            list(msi.get("on_wait") or [])
                        msi["on_update"] = list(msi.get("on_update") or []) + \
                            list(lsi.get("on_update") or [])
                        pending_ld = None
                    else:
                        out.append(pending_ld)
                        pending_ld = None
                out.append(ins)
            if pending_ld is not None:
                out.append(pending_ld)
            blk["instructions"] = out

    ctr = 0
    for fn in m["functions"]:
        for blk in fn["blocks"]:
            out = []
            for ins in blk["instructions"]:
                si = ins.get("sync_info") or {}
                waits = list(si.get("on_wait") or [])
                cap = 2 if ins.get("opcode") == "EventSemaphore" else 1
                while len(waits) > cap:
                    take, waits = waits[:2], waits[2:]
                    ctr += 1
                    out.append({
                        "engine": ins["engine"],
                        "ins": [], "outs": [],
                        "name": f"waitsplit-{ctr}",
                        "opcode": "EventSemaphore",
                        "sync_info": {"on_update": [], "on_wait": take},
                    })
                if si:
                    si["on_wait"] = waits
                out.append(ins)
            blk["instructions"] = out
    return _json.dumps(m).encode()


def _patch_walrus_flags():
    """Run walrus with --enable-ldw-opt=true (requires self-loading
    matmuls, see _legalize_bir) so weight loads target the background
    weight buffer and overlap in-flight matmuls."""
    import concourse.bass_utils as _bu
    if getattr(_bu.run_command, "_ldwopt_patched", False):
        return
    _orig = _bu.run_command

    def _patched(cmd, **kw):
        if isinstance(cmd, list):
            cmd = ['--enable-ldw-opt=true' if c == '--enable-ldw-opt=false'
                   else c for c in cmd]
        return _orig(cmd, **kw)

    _patched._ldwopt_patched = True
    _bu.run_command = _patched


def _get_nc():
    global _CACHED_NC
    if _CACHED_NC is None:
        _patch_walrus_flags()
        nc = _build_nc()
        orig = nc.to_json_bytes
        nc.to_json_bytes = lambda: _legalize_bir(orig())
        _CACHED_NC = nc
    return _CACHED_NC


def kernel(anchor_embeddings, positive_embeddings, labels):
    global last_results
    a = np.asarray(anchor_embeddings, dtype=np.float32)
    p = np.asarray(positive_embeddings, dtype=np.float32)
    l = np.asarray(labels)
    N = a.shape[0]
    assert N == NCOLS and a.shape[1] == D

    idx = np.flatnonzero(l == 1)
    K = int(idx.size)
    count = K * (N - 1)
    if K == 0:
        return np.asarray(0.0, dtype=np.float32)

    # host-side O(N*D) prep: norms, per-row scales, augmentation
    ae = a + np.float32(EPS)
    ae64 = ae.astype(np.float64)
    p64 = p.astype(np.float64)
    a2 = (ae64 * ae64).sum(1)
    p2 = (p64 * p64).sum(1)
    pos_sq = a2 + p2 - 2.0 * (ae64 * p64).sum(1)
    c_all = np.sqrt(np.maximum(pos_sq, 1e-12)) + MARGIN          # f64 [N]

    s2 = np.float64(np.sqrt(2.0))
    phatT = np.empty((KAUG, NCOLS), dtype=np.float16)
    phatT[:D] = (s2 * p64).T.astype(np.float16)
    phatT[D] = np.float16(1.0)
    phatT[D + 1] = p2.astype(np.float16)

    nc = _get_nc()
    total = 0.0
    # device launches cover row chunks; a small remainder (< HOST_TAIL_MAX)
    # is cheaper on the host than another full kernel launch
    chunks = []
    pos = 0
    while K - pos > HOST_TAIL_MAX:
        take = min(ROW_CAP, K - pos)
        chunks.append(idx[pos:pos + take])
        pos += take
    tail_rows = idx[pos:]

    for rows in chunks:
        nrows = rows.size
        # ahat'' rows: (c^2 e_const - ahat) / (2c) so the PE emits
        # y = (c^2 - sq)/(2c) directly; padded rows are zero -> y = 0
        # -> u = sqrt(1) = 1 (ignored by the host reduction anyway)
        ahat_rows = np.zeros((ROW_CAP, KAUG), dtype=np.float16)
        cr = c_all[rows][:, None]
        ahat_rows[:nrows, :D] = (s2 * ae64[rows] / (2.0 * cr)).astype(np.float16)
        ahat_rows[:nrows, D] = ((cr * cr - a2[rows][:, None]) /
                                (2.0 * cr)).ravel().astype(np.float16)
        ahat_rows[:nrows, D + 1] = (-1.0 / (2.0 * cr)).ravel().astype(np.float16)
        # per-row ACT scale -2/c (f32); padded rows get -2 (y=0 there)
        s_pad = np.full(ROW_CAP, -2.0, dtype=np.float32)
        s_pad[:nrows] = (-2.0 / c_all[rows]).astype(np.float32)

        in_maps = []
        for core in range(N_CORES):
            sl = slice(core * R_PER_CORE, (core + 1) * R_PER_CORE)
            in_maps.append({
                "ahat": np.ascontiguousarray(ahat_rows[sl].T),
                "phat": phatT,
                "svec": np.ascontiguousarray(
                    s_pad[sl].reshape(N_ROW_TILES, ROW_TILE).T),
            })

        res = run_bass_kernel_spmd(nc, in_maps, core_ids=list(range(N_CORES)),
                                   trace=TRACE, trace_cores=TRACE_CORES)
        last_results = res

        for core in range(N_CORES):
            acc = res.results[core]["acc"].astype(np.float64)   # [128, N_ACC]
            acc[:, 0] += acc[:, NSLOTS]            # first tile, second half
            acc[:, NSLOTS - 1] += acc[:, NSLOTS + 1]  # last tile, second half
            # ACT-offloaded tile: sum min(u,1) = sum u - sum relu(u-1)
            acc[:, ACT_SLOT] -= acc[:, NSLOTS + 2]
            acc = acc[:, :NSLOTS]
            msum = acc.reshape(ROW_TILE, N_ROW_TILES, N_COL_CHUNKS).sum(-1)
            msum = msum.T.reshape(-1)            # [512] sum_j min(u_ij, 1)
            nreal = max(0, min(R_PER_CORE, nrows - core * R_PER_CORE))
            if nreal == 0:
                continue
            rows_c = rows[core * R_PER_CORE: core * R_PER_CORE + nreal]
            # sum_j relu(c_i - d_ij) = c_i * (N - sum_j min(u_ij, 1))
            total += (c_all[rows_c] * (N - msum[:nreal])).sum()

    if tail_rows.size:
        sq_t = (a2[tail_rows][:, None] + p2[None, :]
                - 2.0 * (ae64[tail_rows] @ p64.T))
        d_t = np.sqrt(np.maximum(sq_t, 1e-12))
        total += np.maximum(c_all[tail_rows][:, None] - d_t, 0.0).sum()

    total -= K  # diagonal j==i contributes exactly MARGIN per label-1 row

    loss = total / count
    return np.asarray(loss, dtype=np.float32)


# revision 6
# speedup vs baseline: 1.0342x; 1.0342x over previous
"""N-pairs custom loss on 8 Trainium2 NeuronCores.

Math
----
reference computes, with a' = anchor + 1e-6:
    sq[i,j] = ||a'_i||^2 + ||p_j||^2 - 2 a'_i . p_j
    dist    = sqrt(max(sq, 1e-12))
    hinge   = relu(diag(dist)[i] + 1 - dist[i,j])
    loss    = sum over {i : label_i == 1, j != i} hinge / count

Device-side restructuring:
  * Only rows with label==1 contribute -> host compacts those rows
    (K ~ N/2), sharded 512 rows/core across 8 cores; a small remainder
    beyond full launches is summed on the host.  512 rows/core keeps the
    PE weight reloads at 4/core (one per row tile) -- finer row sharding
    stalls the PE ~500ns per extra weight swap.
  * The PE emits y = (c_i^2 - sq_ij) / (2 c_i) directly (c_i =
    pos_dist_i + MARGIN): the augmented anchor rows are affinely
    transformed on the host against phat_j = [sqrt(2) p_j, 1, ||p_j||^2]
    (66 contraction dims).  Then u = dist/c = sqrt(1 - 2y/c), so ACT
    computes u = sqrt(y * (-2/c_i) + 1) in one PSUM->SBUF bf16 pass with
    per-partition scale.
  * The DVE reduces sum_j min(u, 1) per row with one tensor_scalar
    (op0=min 1.0 computes the elements, op1=add is the accumulator's
    reduce op).  One tile's reduce runs on ACT instead
    (min(u,1) = u - relu(u-1), two accumulating ACT passes): the DVE
    reduce cadence (~2330ns/tile) is slightly above ACT's (~2060ns), so
    one ACT-offloaded tile levels the two queues.  The diagonal j==i
    contributes exactly MARGIN per row; the host subtracts it.
  * Per-core partials land in a small [128, 20] accumulator; its first
    half is DMA'd out mid-kernel so the final transfer is tiny.
    Host: total = sum_i c_i*(N - msum_i) - K; loss = total/count.

This walrus build accepts only ONE sync wait per instruction; a
post-serialization pass splits excess waits into EventSemaphore
instructions and fuses Ldweights into self-loading matmuls so walrus's
LDW optimization can pipeline weight loads (see _legalize_bir).
"""

import numpy as np

import concourse.bass as bass
import concourse.mybir as mybir
from concourse import tile
from concourse.bass_utils import run_bass_kernel_spmd

N_CORES = 8
NCOLS = 8192              # number of positive embeddings (full N)
D = 64
KAUG = D + 2              # augmented contraction dim
ROW_TILE = 128
N_ROW_TILES = 4
R_PER_CORE = ROW_TILE * N_ROW_TILES      # 512
ROW_CAP = N_CORES * R_PER_CORE           # 4096 label-1 rows per launch
HOST_TAIL_MAX = 256   # rows beyond full launches handled on host (numpy)
COL_CHUNK = 2048                         # 4 PSUM banks
N_COL_CHUNKS = NCOLS // COL_CHUNK        # 4
MM_FREE = 512                            # moving free dim per matmul (PSUM bank)
MARGIN = 1.0
EPS = 1e-6
ACT_TILE = (3, 2)    # the (row-tile, chunk) whose reduce runs on ACT
ACT_SLOT = ACT_TILE[0] * N_COL_CHUNKS + ACT_TILE[1]
NSLOTS = N_ROW_TILES * N_COL_CHUNKS      # 16
N_ACC = NSLOTS + 3   # +2 split-tile halves, +1 ACT tile's relu term
ACC_EARLY = 8        # acc columns DMA'd out mid-kernel

_CACHED_NC = None
last_results = None       # BassKernelResults of the most recent launch
TRACE = False             # set True (e.g. from test.py) to capture a profile
TRACE_CORES = None        # e.g. list(range(8)) to profile all cores


def _build_nc():
    nc = bass.Bass()
    ahat = nc.dram_tensor("ahat", [KAUG, R_PER_CORE], mybir.dt.float16,
                          kind="ExternalInput")
    phat = nc.dram_tensor("phat", [KAUG, NCOLS], mybir.dt.float16,
                          kind="ExternalInput")
    svec = nc.dram_tensor("svec", [ROW_TILE, N_ROW_TILES], mybir.dt.float32,
                          kind="ExternalInput")
    acc_out = nc.dram_tensor("acc", [ROW_TILE, N_ACC],
                             mybir.dt.float32, kind="ExternalOutput")

    with tile.TileContext(nc) as tc:
        with (
            tc.tile_pool(name="const", bufs=1) as const_pool,
            # one SBUF slot per tile's u: slot reuse would force extra
            # ACT/DVE WAR semaphore waits (each costing a split EVSEM)
            tc.tile_pool(name="upool", bufs=NSLOTS + 2) as u_pool,
            tc.tile_pool(name="psum", bufs=2, space="PSUM") as psum_pool,
        ):
            ahat_sb = const_pool.tile([KAUG, R_PER_CORE], mybir.dt.float16)
            phat_sb = const_pool.tile([KAUG, NCOLS], mybir.dt.float16)
            s_sb = const_pool.tile([ROW_TILE, N_ROW_TILES], mybir.dt.float32)
            negone_sb = const_pool.tile([ROW_TILE, 1], mybir.dt.float32)
            junk_sb = const_pool.tile([ROW_TILE, COL_CHUNK], mybir.dt.bfloat16)
            junk2_sb = const_pool.tile([ROW_TILE, COL_CHUNK], mybir.dt.bfloat16)
            acc_sb = const_pool.tile([ROW_TILE, N_ACC], mybir.dt.float32)
            nc.vector.memset(negone_sb[:], -1.0)

            # preload the sqrt activation-table set (~2.7us) as early as
            # possible on the ACT engine: scale=0.0 means the input value
            # is never consumed, so the only dependency is the tiny negone
            # memset and the table load overlaps the NEFF preamble / DMA
            # window instead of gating the first real sqrt
            warm_out = const_pool.tile([1, 1], mybir.dt.float16)
            nc.scalar.activation(warm_out[:], negone_sb[:1, :1],
                                 mybir.ActivationFunctionType.Sqrt,
                                 scale=0.0)

            # Three DMA queues (sync + scalar are both HWDGE rings, gpsimd
            # is SWDGE).  Pieces are ordered per queue in compute order;
            # the scalar queue is busy early with the activation-table
            # load, so it only carries a slack mid-kernel piece.
            nc.sync.dma_start(ahat_sb[:], ahat[:])
            nc.sync.dma_start(phat_sb[:, 0:512], phat[:, 0:512])
            nc.gpsimd.dma_start(s_sb[:], svec[:])
            nc.sync.dma_start(phat_sb[:, 512:1024], phat[:, 512:1024])
            nc.gpsimd.dma_start(phat_sb[:, 1024:2048], phat[:, 1024:2048])
            nc.gpsimd.dma_start(phat_sb[:, 2048:3072], phat[:, 2048:3072])
            nc.gpsimd.dma_start(phat_sb[:, 3072:4096], phat[:, 3072:4096])
            nc.sync.dma_start(phat_sb[:, 4096:5120], phat[:, 4096:5120])
            nc.scalar.dma_start(phat_sb[:, 5120:6144], phat[:, 5120:6144])
            nc.sync.dma_start(phat_sb[:, 6144:7168], phat[:, 6144:7168])
            nc.gpsimd.dma_start(phat_sb[:, 7168:8192], phat[:, 7168:8192])

            def emit_tile(r, col0, clen, slot, on_act=False):
                ps = psum_pool.tile([ROW_TILE, COL_CHUNK], mybir.dt.float32,
                                    tag="ps")
                for k in range(clen // MM_FREE):
                    nc.tensor.matmul(
                        ps[:, k * MM_FREE:(k + 1) * MM_FREE],
                        ahat_sb[:, r * ROW_TILE:(r + 1) * ROW_TILE],
                        phat_sb[:, col0 + k * MM_FREE:col0 + (k + 1) * MM_FREE],
                        start=True, stop=True,
                    )
                u_t = u_pool.tile([ROW_TILE, COL_CHUNK],
                                  mybir.dt.bfloat16, tag="u")
                # u = sqrt(y * (-2/c_i) + 1) = dist/c; for the ACT-offloaded
                # tile also accumulate sum(u) in the same pass
                nc.scalar.activation(u_t[:, :clen], ps[:, :clen],
                                     mybir.ActivationFunctionType.Sqrt,
                                     scale=s_sb[:, r:r + 1], bias=1.0,
                                     accum_out=(acc_sb[:, slot:slot + 1]
                                                if on_act else None))
                if on_act:
                    # min(u,1) = u - relu(u-1) exactly, so this tile's sum
                    # needs only a second ACT pass (relu with bias -1 and
                    # accumulate); the DVE-side reduce is the kernel's
                    # bottleneck so one tile moves engines to balance.
                    nc.scalar.activation(junk2_sb[:, :clen], u_t[:, :clen],
                                         mybir.ActivationFunctionType.Relu,
                                         bias=negone_sb[:, 0:1],
                                         accum_out=acc_sb[:, N_ACC - 1:N_ACC])
                else:
                    # accum_out[p] = sum_j min(u, 1): op0 computes the out
                    # elements, op1 is the accumulator's reduce op
                    nc.vector.tensor_scalar(
                        out=junk_sb[:, :clen],
                        in0=u_t[:, :clen],
                        scalar1=1.0, scalar2=0.0,
                        op0=mybir.AluOpType.min,
                        op1=mybir.AluOpType.add,
                        accum_out=acc_sb[:, slot:slot + 1],
                    )

            nc.vector.memset(acc_sb[:], 0.0)
            for r in range(N_ROW_TILES):
                for t in range(N_COL_CHUNKS):
                    slot = r * N_COL_CHUNKS + t
                    first = (r == 0 and t == 0)
                    last = (r == N_ROW_TILES - 1 and t == N_COL_CHUNKS - 1)
                    if first:
                        # 512-wide head tile: starts after one DMA piece
                        emit_tile(r, 0, 512, slot)
                        emit_tile(r, 512, COL_CHUNK - 512, NSLOTS)
                    elif last:
                        # 512-wide final tile: shorter end-of-pipeline drain
                        h = COL_CHUNK - 512
                        emit_tile(r, t * COL_CHUNK, h, slot)
                        emit_tile(r, t * COL_CHUNK + h, 512, NSLOTS + 1)
                    else:
                        emit_tile(r, t * COL_CHUNK, COL_CHUNK, slot,
                                  on_act=((r, t) == ACT_TILE))
                    if slot == 11 and t == 3:
                        # first 8 acc columns (row tiles 0-1) are final by
                        # now: ship them early so the tail transfer is tiny
                        nc.scalar.dma_start(acc_out[:, 0:ACC_EARLY],
                                            acc_sb[:, 0:ACC_EARLY])
            nc.sync.dma_start(acc_out[:, ACC_EARLY:N_ACC],
                              acc_sb[:, ACC_EARLY:N_ACC])
    return nc


def _legalize_bir(bir_bytes):
    """Two fixups on the serialized BIR before walrus:

    1. Fuse each standalone Ldweights into its paired (self-loading)
       Matmult: walrus's LDW optimization (background weight buffer ->
       weight loads overlap in-flight matmuls) rejects standalone
       InstLdweights, and without it every LDW/MM pair serializes at the
       full matmul drain latency (~630ns instead of ~430ns per matmul).

    2. This walrus build accepts only ONE sync wait per instruction (two
       on EventSemaphore); Tile emits more on some (epilogue drain, ...).
       Split excess waits into standalone EventSemaphore wait instructions
       on the same engine, inserted immediately before (semantically
       identical: the engine blocks on the same condition set, in order).
    """
    import json as _json
    m = _json.loads(bir_bytes)
    for fn in m["functions"]:
        for blk in fn["blocks"]:
            out = []
            pending_ld = None
            for ins in blk["instructions"]:
                op = ins.get("opcode")
                if op == "Ldweights":
                    if pending_ld is not None:
                        out.append(pending_ld)
                    pending_ld = ins
                    continue
                if op == "Matmult" and pending_ld is not None:
                    if pending_ld["ins"][0] == ins["ins"][1]:
                        ins["ldweights"] = True
                        lsi = pending_ld.get("sync_info") or {}
                        msi = ins.setdefault("sync_info", {})
                        msi["on_wait"] = list(lsi.get("on_wait") or []) + \
                            list(msi.get("on_wait") or [])
                        msi["on_update"] = list(msi.get("on_update") or []) + \
                            list(lsi.get("on_update") or [])
                        pending_ld = None
                    else:
                        out.append(pending_ld)
                        pending_ld = None
                out.append(ins)
            if pending_ld is not None:
                out.append(pending_ld)
            blk["instructions"] = out

    ctr = 0
    for fn in m["functions"]:
        for blk in fn["blocks"]:
            out = []
            for ins in blk["instructions"]:
                si = ins.get("sync_info") or {}
                waits = list(si.get("on_wait") or [])
                cap = 2 if ins.get("opcode") == "EventSemaphore" else 1
                while len(waits) > cap:
                    take, waits = waits[:2], waits[2:]
                    ctr += 1
                    out.append({
                        "engine": ins["engine"],
                        "ins": [], "outs": [],
                        "name": f"waitsplit-{ctr}",
                        "opcode": "EventSemaphore",
                        "sync_info": {"on_update": [], "on_wait": take},
                    })
                if si:
                    si["on_wait"] = waits
                out.append(ins)
            blk["instructions"] = out
    return _json.dumps(m).encode()


def _patch_walrus_flags():
    """Run walrus with --enable-ldw-opt=true (requires self-loading
    matmuls, see _legalize_bir) so weight loads target the background
    weight buffer and overlap in-flight matmuls."""
    import concourse.bass_utils as _bu
    if getattr(_bu.run_command, "_ldwopt_patched", False):
        return
    _orig = _bu.run_command

    def _patched(cmd, **kw):
        if isinstance(cmd, list):
            cmd = ['--enable-ldw-opt=true' if c == '--enable-ldw-opt=false'
                   else c for c in cmd]
        return _orig(cmd, **kw)

    _patched._ldwopt_patched = True
    _bu.run_command = _patched


def _get_nc():
    global _CACHED_NC
    if _CACHED_NC is None:
        _patch_walrus_flags()
        nc = _build_nc()
        orig = nc.to_json_bytes
        nc.to_json_bytes = lambda: _legalize_bir(orig())
        _CACHED_NC = nc
    return _CACHED_NC


def kernel(anchor_embeddings, positive_embeddings, labels):
    global last_results
    a = np.asarray(anchor_embeddings, dtype=np.float32)
    p = np.asarray(positive_embeddings, dtype=np.float32)
    l = np.asarray(labels)
    N = a.shape[0]
    assert N == NCOLS and a.shape[1] == D

    idx = np.flatnonzero(l == 1)
    K = int(idx.size)
    count = K * (N - 1)
    if K == 0:
        return np.asarray(0.0, dtype=np.float32)

    # host-side O(N*D) prep: norms, per-row scales, augmentation
    ae = a + np.float32(EPS)
    ae64 = ae.astype(np.float64)
    p64 = p.astype(np.float64)
    a2 = (ae64 * ae64).sum(1)
    p2 = (p64 * p64).sum(1)
    pos_sq = a2 + p2 - 2.0 * (ae64 * p64).sum(1)
    c_all = np.sqrt(np.maximum(pos_sq, 1e-12)) + MARGIN          # f64 [N]

    s2 = np.float64(np.sqrt(2.0))
    phatT = np.empty((KAUG, NCOLS), dtype=np.float16)
    phatT[:D] = (s2 * p64).T.astype(np.float16)
    phatT[D] = np.float16(1.0)
    phatT[D + 1] = p2.astype(np.float16)

    nc = _get_nc()
    total = 0.0
    # device launches cover row chunks; a small remainder (< HOST_TAIL_MAX)
    # is cheaper on the host than another full kernel launch
    chunks = []
    pos = 0
    while K - pos > HOST_TAIL_MAX:
        take = min(ROW_CAP, K - pos)
        chunks.append(idx[pos:pos + take])
        pos += take
    tail_rows = idx[pos:]

    for rows in chunks:
        nrows = rows.size
        # ahat'' rows: (c^2 e_const - ahat) / (2c) so the PE emits
        # y = (c^2 - sq)/(2c) directly; padded rows are zero -> y = 0
        # -> u = sqrt(1) = 1 (ignored by the host reduction anyway)
        ahat_rows = np.zeros((ROW_CAP, KAUG), dtype=np.float16)
        cr = c_all[rows][:, None]
        ahat_rows[:nrows, :D] = (s2 * ae64[rows] / (2.0 * cr)).astype(np.float16)
        ahat_rows[:nrows, D] = ((cr * cr - a2[rows][:, None]) /
                                (2.0 * cr)).ravel().astype(np.float16)
        ahat_rows[:nrows, D + 1] = (-1.0 / (2.0 * cr)).ravel().astype(np.float16)
        # per-row ACT scale -2/c (f32); padded rows get -2 (y=0 there)
        s_pad = np.full(ROW_CAP, -2.0, dtype=np.float32)
        s_pad[:nrows] = (-2.0 / c_all[rows]).astype(np.float32)

        in_maps = []
        for core in range(N_CORES):
            sl = slice(core * R_PER_CORE, (core + 1) * R_PER_CORE)
            in_maps.append({
                "ahat": np.ascontiguousarray(ahat_rows[sl].T),
                "phat": phatT,
                "svec": np.ascontiguousarray(
                    s_pad[sl].reshape(N_ROW_TILES, ROW_TILE).T),
            })

        res = run_bass_kernel_spmd(nc, in_maps, core_ids=list(range(N_CORES)),
                                   trace=TRACE, trace_cores=TRACE_CORES)
        last_results = res

        for core in range(N_CORES):
            acc = res.results[core]["acc"].astype(np.float64)   # [128, N_ACC]
            acc[:, 0] += acc[:, NSLOTS]            # first tile, second half
            acc[:, NSLOTS - 1] += acc[:, NSLOTS + 1]  # last tile, second half
            # ACT-offloaded tile: sum min(u,1) = sum u - sum relu(u-1)
            acc[:, ACT_SLOT] -= acc[:, NSLOTS + 2]
            acc = acc[:, :NSLOTS]
            msum = acc.reshape(ROW_TILE, N_ROW_TILES, N_COL_CHUNKS).sum(-1)
            msum = msum.T.reshape(-1)            # [512] sum_j min(u_ij, 1)
            nreal = max(0, min(R_PER_CORE, nrows - core * R_PER_CORE))
            if nreal == 0:
                continue
            rows_c = rows[core * R_PER_CORE: core * R_PER_CORE + nreal]
            # sum_j relu(c_i - d_ij) = c_i * (N - sum_j min(u_ij, 1))
            total += (c_all[rows_c] * (N - msum[:nreal])).sum()

    if tail_rows.size:
        sq_t = (a2[tail_rows][:, None] + p2[None, :]
                - 2.0 * (ae64[tail_rows] @ p64.T))
        d_t = np.sqrt(np.maximum(sq_t, 1e-12))
        total += np.maximum(c_all[tail_rows][:, None] - d_t, 0.0).sum()

    total -= K  # diagonal j==i contributes exactly MARGIN per label-1 row

    loss = total / count
    return np.asarray(loss, dtype=np.float32)
